# revision 64
# baseline (speedup 1.0000x reference)
"""BrainAgeGAT Trainium2 kernel: 2-layer GATv2 + mean-pool + MLP on 8 NeuronCores.

Strategy (per sharding_hint: shard edges; 1D-shard nodes; all-reduce pool):
  - Edges (incl. self loops) are sorted by destination and sharded by
    destination-node range across the 8 cores, so each core owns the full
    softmax/scatter for its destination nodes.
  - Per-core destination nodes are packed into blocks of <=127 "slots"
    (slot 127 of each 128-row block is a garbage slot).
  - Node transforms xl = x@Wl / xr = x@Wr are computed on each core for its
    own node shard; the xl table is AllGather'd so every core can gather any
    source row. Per edge, 512-byte bf16 xl rows are fetched with dma_gather
    (SWDGE, int16 indices; the 51200-row table is split in two halves to
    stay within int16). Padding indices are negative => skipped by SWDGE.
  - xr[dst] is NOT gathered: per edge tile, u = xl[src]+xr[dst] is built in
    PSUM by two TensorE matmuls: (slot->edge one-hot) @ xr_block, plus
    identity @ gathered-xl. Both one-hot orientations (scatter [edge->slot]
    and gather [slot->edge]) are host-precomputed bf16 strips, DMA'd per
    block; padding edges get all-zero one-hot columns.
  - logits = per-head tree-reduction of att * leaky_relu(u) (ACT Prelu from
    PSUM + DVE); softmax needs no max subtraction at these magnitudes.
  - Message = exp(logit) * xl[src] (head-broadcast on DVE); the per-head
    exp is appended as 8 extra columns so ONE one-hot scatter matmul per
    tile accumulates both the message sum and the softmax denominator.
  - Layer-2 node tables are computed inside the layer-1 loop from the
    freshly transposed h tiles (no DRAM round-trip); xr tables stay
    SBUF-resident.
  - Mean-pool uses per-block one-hot graph-selector matmuls into a
    persistent PSUM accumulator, an 8-core AllReduce, and a tiny MLP.
"""

import math
import sys

sys.path.insert(0, "/opt/trn_rl_repo")

import ml_dtypes
import numpy as np

import concourse.bacc as bacc
import concourse.bass as bass
import concourse.mybir as mybir
import concourse.tile as tile
from concourse import library_config
from concourse.vector_clock import ScopedClock

BF16 = ml_dtypes.bfloat16
FP8 = ml_dtypes.float8_e4m3

# ---------------------------------------------------------------------------
# Patches for walrus' one-sync-wait-per-instruction limit.
# ---------------------------------------------------------------------------


def _drain_and_barrier(self, tick_clock, wait_clock):
    nc = self.nc
    probe = nc.sync.nop(nofuse=True, hint="drain_wait_split")
    wait_clock.add_sem_waits(probe.ins, ScopedClock({None: tick_clock.global_clock}))
    si = probe.ins.sync_info
    waits = list(si.on_wait) if si and si.on_wait else []
    if len(waits) > 1:
        si.on_wait = waits[:1]
        for w in waits[1:]:
            extra = nc.sync.nop(nofuse=True, hint="drain_wait_split")
            extra.ins.sync_info = type(si)(on_wait=[w], on_update=[])
    nc.sync.drain()
    nc.all_engine_barrier()
    assert self.sems is not None
    popped = nc._tile_sem_poison_stack.pop()
    assert popped is self._sem_poison
    nc.clear_and_free_semaphores(list(self.sems.allocated().values()))
    nc.all_engine_barrier()


tile.TileContext._drain_and_barrier = _drain_and_barrier


def _split_waits(nc):
    """walrus codegen accepts one sync-wait command per instruction; Tile can
    emit several. Hoist extras onto preceding same-engine NoOps."""
    for bb in nc.main_func.blocks:
        out = []
        for ins in bb.instructions:
            si = ins.sync_info
            waits = list(si.on_wait) if si and si.on_wait else []
            if len(waits) > 1:
                for w in waits[:-1]:
                    nop = mybir.InstNoOp(
                        name=nc.get_next_instruction_name(), ins=[], outs=[]
                    )
                    nop.engine = ins.engine
                    nop.sync_info = mybir.SyncInfo(on_wait=[w], on_update=[])
                    nc.register_instruction(nop)
                    out.append(nop)
                si.on_wait = [waits[-1]]
            out.append(ins)
        bb.instructions = out


# ---------------------------------------------------------------------------
# Model dimensions (hardcoded per problem spec)
# ---------------------------------------------------------------------------
N = 50000
E = 800000
G = 128
H = 8
C = 32
HC = H * C  # 256
HCD = HC + H  # message + per-head denominator columns
P = 128
NCORES = 8
SLOTS = 127  # real slots per block (slot 127 = garbage)
MAXI16 = 25600  # table-piece size for int16 gather indices
CH = 5  # tiles per dma_gather call (640 rows; larger overflows the SWDGE ring)


class Cfg:
    """tba/tbb: per-block tile counts for the two xl-table pieces (uniform
    across cores so the SPMD program is identical)."""

    def __init__(self, n_nodes, ncores, nblk, tba, tbb, ucnt=None):
        self.n_nodes = n_nodes
        self.ncores = ncores
        self.nodes_pc = n_nodes // ncores
        self.nblk = nblk
        self.cap = nblk * P
        self.capext = ncores * self.cap
        self.tba = tba  # list[nblk]
        self.tbb = tbb  # list[nblk]
        self.tb = [a + b for a, b in zip(tba, tbb)]
        self.tbmax = max(self.tb)
        self.ttot = sum(self.tb)
        self.col0 = np.concatenate([[0], np.cumsum(self.tb)]).astype(int)
        self.ncalls = sum(
            -(-a // CH) + -(-bb // CH) for a, bb in zip(tba, tbb)
        )
        self.ucnt = ucnt
        self.npiece = 2 if self.capext > MAXI16 else 1
        if self.npiece == 1:
            assert all(b == 0 for b in tbb)


# ---------------------------------------------------------------------------
# Host-side preprocessing
# ---------------------------------------------------------------------------


def _f32(a):
    return np.ascontiguousarray(a, dtype=np.float32)


def _bf(a):
    return np.ascontiguousarray(np.asarray(a, dtype=np.float32).astype(BF16))


def _wrap_idx(ids):
    """Gather-index list -> [128, len/16] int16 in the SWDGE wrap layout
    (idx j read from [j % 16, j // 16], replicated over the 8 Q7 cores)."""
    ids = np.asarray(ids, np.int16)
    assert len(ids) % 16 == 0
    w = ids.reshape(-1, 16).T  # [16, s]
    return np.tile(w, (8, 1))  # [128, s]


def _assign_nodes(src, dst, n_nodes, ncores, nblk):
    """Assign each node to a (core, block, slot) so per-(core,block) edge
    counts are balanced per table piece. Even nodes -> cores 0..3 (piece A
    rows), odd -> cores 4..7, so an edge's piece is fixed by src parity.
    Returns node2row[v] = core*cap + block*P + slot."""
    cap = nblk * P
    deg_tot = np.bincount(dst, minlength=n_nodes)
    degA = np.bincount(dst[src % 2 == 0], minlength=n_nodes)
    degB = deg_tot - degA

    half_cores = ncores // 2
    npc = n_nodes // ncores
    caps_blk = np.minimum(npc - SLOTS * np.arange(nblk), SLOTS)  # per-block slots
    node2row = np.zeros(n_nodes, np.int64)
    for half in (0, 1):
        nodes = np.where(np.arange(n_nodes) % 2 == half)[0]
        nodes = nodes[np.argsort(-(deg_tot[nodes]))]
        nbins = half_cores * nblk
        binA = np.zeros(nbins)
        binB = np.zeros(nbins)
        fill = np.zeros(nbins, np.int64)
        caps = np.tile(caps_blk, half_cores).astype(np.int64)  # bin = c*nblk+b
        rowbase = np.repeat(
            np.arange(half * half_cores, half * half_cores + half_cores) * cap,
            nblk,
        ) + np.tile(np.arange(nblk) * P, half_cores)
        for v in nodes:
            score = np.maximum(binA + degA[v], binB + degB[v])
            score[fill >= caps] = np.inf
            i = int(np.argmin(score))
            node2row[v] = rowbase[i] + fill[i]
            binA[i] += degA[v]
            binB[i] += degB[v]
            fill[i] += 1
    return node2row


def _plan_blocks(edge_index, n_nodes, ncores):
    """Assign nodes, bucket edges; return per-core edge structures + uniform
    tile counts + per-call gather counts."""
    npc = n_nodes // ncores
    nblk = (npc + SLOTS - 1) // SLOTS
    cap = nblk * P
    capext = ncores * cap
    npiece = 2 if capext > MAXI16 else 1

    src = np.concatenate([edge_index[0], np.arange(n_nodes)]).astype(np.int64)
    dst = np.concatenate([edge_index[1], np.arange(n_nodes)]).astype(np.int64)

    node2row = _assign_nodes(src, dst, n_nodes, ncores, nblk)
    srow = node2row[src]
    drow = node2row[dst]
    order = np.argsort(drow, kind="stable")
    srow, drow = srow[order], drow[order]

    percore = []
    na = np.zeros((ncores, nblk), int)
    nb_ = np.zeros((ncores, nblk), int)
    ucnt = []  # per-gather-call uniform real-index count (max over cores)
    for c in range(ncores):
        sel = (drow >= c * cap) & (drow < (c + 1) * cap)
        bsrow = srow[sel]
        loc = drow[sel] - c * cap
        blocks = []
        for b in range(nblk):
            es = (loc // P) == b
            rs = bsrow[es]
            slots = (loc[es] % P).astype(np.int64)
            piece = rs // MAXI16 if npiece == 2 else np.zeros_like(rs)
            a_i = np.where(piece == 0)[0]
            b_i = np.where(piece == 1)[0]
            blocks.append((rs, slots, a_i, b_i))
            na[c, b] = len(a_i)
            nb_[c, b] = len(b_i)
        percore.append(blocks)
    # uniform per-block tile counts (max over cores per piece)
    tba = [int(math.ceil(na[:, b].max() / P)) for b in range(nblk)]
    tbb = [int(math.ceil(nb_[:, b].max() / P)) if npiece == 2 else 0
           for b in range(nblk)]
    # uniform per-call real-index counts, in device call order (per block:
    # A then B), rounded up to multiples of 64 to bound distinct constants
    for b in range(nblk):
        for n_real, ntile in ((int(na[:, b].max()), tba[b]),
                              (int(nb_[:, b].max()), tbb[b])):
            for q0 in range(0, ntile, CH):
                q1 = min(q0 + CH, ntile)
                u = min(max(n_real - q0 * P, 1), (q1 - q0) * P)
                ucnt.append(int(min(-(-u // 64) * 64, (q1 - q0) * P)))
    return percore, nblk, tba, tbb, ucnt, node2row


def _prep(x, edge_index, batch, u, weights, cfg: Cfg, percore, node2row):
    npc = cfg.nodes_pc
    att1 = weights["att1"]
    att2 = weights["att2"]

    def att_rep(att):
        return _bf(np.broadcast_to(att.reshape(-1), (P, HC)))

    arange_p = np.arange(P, dtype=np.int64)
    maps = []
    for c in range(cfg.ncores):
        m = {}
        ixa = np.zeros((P, max(sum(cfg.tba), 1) * 8), np.int16)
        ixb = np.zeros((P, max(sum(cfg.tbb), 1) * 8), np.int16)
        ot = np.zeros((P, cfg.ttot * P), FP8)   # scatter one-hot [edge, slot]
        ott = np.zeros((P, cfg.ttot * P), FP8)  # gather one-hot [slot, edge]
        call_i = 0
        ca = cb = 0
        for b in range(cfg.nblk):
            rs, slots, a_i, b_i = percore[c][b]
            na, nb_ = len(a_i), len(b_i)
            ea = cfg.tba[b] * P
            eb = cfg.tbb[b] * P
            # gather indices (relative to piece). Per chunk: real prefix,
            # row-0 pads up to the uniform count ucnt[call], -1 beyond (the
            # Q7 kernel trims trailing negatives; num_idxs_reg == count of
            # non-negative indices keeps ring bookkeeping consistent).
            ia = np.full(ea, -1, np.int64)
            ia[:na] = rs[a_i] % MAXI16
            ib = np.full(eb, -1, np.int64)
            ib[:nb_] = rs[b_i] % MAXI16
            for n_real, arr, ntile in ((na, ia, cfg.tba[b]), (nb_, ib, cfg.tbb[b])):
                for q0 in range(0, ntile, CH):
                    q1 = min(q0 + CH, ntile)
                    uc = cfg.ucnt[call_i]
                    call_i += 1
                    lo = min(max(n_real - q0 * P, 0), (q1 - q0) * P)
                    assert lo <= uc
                    arr[q0 * P + lo : q0 * P + uc] = 0
            if ea:
                ixa[:, ca : ca + cfg.tba[b] * 8] = _wrap_idx(ia)
            if eb:
                ixb[:, cb : cb + cfg.tbb[b] * 8] = _wrap_idx(ib)
            # slot sequence in edge order [A | Apad | B | Bpad]; pad -> -1
            sseq = np.full(ea + eb, -1, np.int64)
            sseq[:na] = slots[a_i]
            sseq[ea : ea + nb_] = slots[b_i]
            S = sseq.reshape(cfg.tb[b], P)  # [t, e]
            M = (S[:, :, None] == arange_p[None, None, :]).astype(FP8)  # [t,e,s]
            c0 = cfg.col0[b] * P
            c1 = cfg.col0[b + 1] * P
            ot[:, c0:c1] = M.transpose(1, 0, 2).reshape(P, -1)
            ott[:, c0:c1] = M.transpose(2, 0, 1).reshape(P, -1)
            ca += cfg.tba[b] * 8
            cb += cfg.tbb[b] * 8
        m["ixa"] = ixa
        m["ixb"] = ixb
        m["ots"] = ot
        m["otts"] = ott

        vs = np.where((node2row >= c * cfg.cap) & (node2row < (c + 1) * cfg.cap))[0]
        rows = node2row[vs] - c * cfg.cap
        xs = np.zeros((cfg.cap, x.shape[1]), np.float32)
        xs[rows] = x[vs]
        m["xT"] = _bf(xs.T)

        gsel = np.zeros((cfg.cap, G), np.float32)
        gsel[rows, np.asarray(batch)[vs]] = 1.0
        m["gsel"] = _bf(gsel)
        maps.append(m)

    counts = np.bincount(np.asarray(batch), minlength=G).astype(np.float32)
    shared = {
        "Wl1": _bf(weights["Wl1"]),
        "Wr1": _bf(weights["Wr1"]),
        "Wl2": _bf(weights["Wl2"]),
        "Wr2": _bf(weights["Wr2"]),
        "att1r": att_rep(att1),
        "att2r": att_rep(att2),
        "b1r": _bf(np.broadcast_to(weights["b1"], (P, HC))),
        "b2r": _bf(np.broadcast_to(weights["b2"], (P, HC))),
        "ident": _bf(np.eye(P, dtype=np.float32)),
        "ident8": np.ascontiguousarray(np.eye(P, dtype=np.float32).astype(FP8)),
        "crecip": _f32((1.0 / np.maximum(counts, 1.0)).reshape(G, 1)),
        "Wlin1": _bf(weights["W_lin1"]),
        "blin1r": _f32(np.broadcast_to(weights["b_lin1"], (G, 64))),
        "Wout": _bf(weights["W_out"]),
        "boutr": _f32(np.full((G, 1), float(weights["b_out"][0]), np.float32)),
        "ub": _bf(u),
    }
    for m in maps:
        m.update(shared)
    return maps


# ---------------------------------------------------------------------------
# Device program
# ---------------------------------------------------------------------------


def _build(cfg: Cfg, in_dim=3):
    dt = mybir.dt
    bf = dt.bfloat16
    f32 = dt.float32
    nc = bacc.Bacc(None)
    groups = [list(range(cfg.ncores))]

    def prm(name, shape, dtype):
        return nc.declare_dram_parameter(name, list(shape), dtype, isOutput=False)

    xT = prm("xT", [in_dim, cfg.cap], bf)
    ixa = prm("ixa", [P, max(sum(cfg.tba), 1) * 8], dt.int16)
    ixb = prm("ixb", [P, max(sum(cfg.tbb), 1) * 8], dt.int16)
    f8 = dt.float8e4
    otsp = prm("ots", [P, cfg.ttot * P], f8)
    ottsp = prm("otts", [P, cfg.ttot * P], f8)
    ident8p = prm("ident8", [P, P], f8)
    Wl1p = prm("Wl1", [in_dim, HC], bf)
    Wr1p = prm("Wr1", [in_dim, HC], bf)
    Wl2p = prm("Wl2", [HC, HC], bf)
    Wr2p = prm("Wr2", [HC, HC], bf)
    att1r = prm("att1r", [P, HC], bf)
    att2r = prm("att2r", [P, HC], bf)
    b1r = prm("b1r", [P, HC], bf)
    b2r = prm("b2r", [P, HC], bf)
    identp = prm("ident", [P, P], bf)
    gselp = prm("gsel", [cfg.cap, G], bf)
    crecip = prm("crecip", [G, 1], f32)
    Wlin1 = prm("Wlin1", [HC, 64], bf)
    blin1r = prm("blin1r", [G, 64], f32)
    Woutp = prm("Wout", [64 + 3, 1], bf)
    boutr = prm("boutr", [G, 1], f32)
    ub = prm("ub", [G, 3], bf)
    out_g = nc.declare_dram_parameter("out_g", [G, 1], f32, isOutput=True)

    with tile.TileContext(nc) as tc:
        with (
            tc.tile_pool(name="const", bufs=1) as constp,
            tc.tile_pool(name="meta", bufs=3) as metap,
            tc.tile_pool(name="strip", bufs=2) as stripp,
            tc.tile_pool(name="gbuf", bufs=5) as gbufp,
            tc.tile_pool(name="work", bufs=2) as workp,
            tc.tile_pool(name="small", bufs=3) as smallp,
            tc.tile_pool(name="psU", bufs=3, space="PSUM") as psU,
            tc.tile_pool(name="psS", bufs=2, space="PSUM") as psS,
            tc.tile_pool(name="psA", bufs=2, space="PSUM") as psA,
            tc.tile_pool(name="psG", bufs=1, space="PSUM") as psG,
            tc.tile_pool(name="dram", bufs=1, space="DRAM") as dram,
        ):
            # ---- constants to SBUF ----
            def cload(p):
                t = constp.tile([p.shape[0], p.shape[1]], p.dtype, name=p.name + "_s")
                nc.sync.dma_start(out=t[:], in_=p[:])
                return t

            def cload_k(p):
                nk = (p.shape[0] + P - 1) // P
                out = []
                for kt in range(nk):
                    rows = slice(kt * P, min((kt + 1) * P, p.shape[0]))
                    t = constp.tile(
                        [rows.stop - rows.start, p.shape[1]], p.dtype,
                        name=f"{p.name}_s{kt}",
                    )
                    nc.sync.dma_start(out=t[:], in_=p[rows, :])
                    out.append(t)
                return out

            xT_s = cload(xT)
            Wl1_s = cload_k(Wl1p)
            Wr1_s = cload_k(Wr1p)
            Wl2_s = cload_k(Wl2p)
            Wr2_s = cload_k(Wr2p)
            att1r_s = cload(att1r)
            att2r_s = cload(att2r)
            b1r_s = cload(b1r)
            b2r_s = cload(b2r)
            ident_s = cload(identp)
            ident8_s = cload(ident8p)
            crecip_s = cload(crecip)
            Wlin1_s = cload_k(Wlin1)
            blin1r_s = cload(blin1r)
            Wout_s = cload(Woutp)
            boutr_s = cload(boutr)
            ub_s = cload(ub)

            # ---- internal DRAM ----
            xl1_own = dram.tile([cfg.cap, HC], bf)
            xl1_ext = dram.tile([cfg.capext, HC], bf, addr_space="Shared")
            xr1_tab = dram.tile([cfg.cap, HC], bf)
            xl2_own = dram.tile([cfg.cap, HC], bf)
            xl2_ext = dram.tile([cfg.capext, HC], bf, addr_space="Shared")
            xr2_tab = dram.tile([cfg.cap, HC], bf)
            gp_in = dram.tile([G, HC], f32)
            gp_out = dram.tile([G, HC], f32, addr_space="Shared")

            A_ = mybir.AluOpType
            AF = mybir.ActivationFunctionType

            # Pre-zero the gather buffers once: padded rows are skipped by
            # SWDGE (negative idx), so stale contents must be finite.
            for _ in range(5):
                g0 = gbufp.tile([P, cfg.tbmax, HC], bf, tag="gxl")
                nc.gpsimd.memset(g0[:], 0.0)

            # ---- layer-1 node tables; xl first so the AllGather starts
            # early, xr computes while the collective runs ----
            def l1_tables(W_s, tab):
                for b in range(cfg.nblk):
                    lt = xT_s[:, b * P : (b + 1) * P]
                    ps = psA.tile([P, HC], f32, tag="a")
                    nc.tensor.matmul(ps[:], lhsT=lt, rhs=W_s[0][:], start=True, stop=True)
                    ev = smallp.tile([P, HC], bf, tag="tabev")
                    nc.scalar.activation(out=ev[:], in_=ps[:], func=AF.Copy)
                    nc.sync.dma_start(out=tab[b * P : (b + 1) * P, :], in_=ev[:])

            l1_tables(Wl1_s, xl1_own)
            nc.gpsimd.collective_compute(
                "AllGather", A_.bypass, replica_groups=groups,
                ins=[xl1_own.opt()], outs=[xl1_ext.opt()],
            )
            l1_tables(Wr1_s, xr1_tab)

            # ================= edge pipeline =================
            def edge_layer(xl_ext, xr_tab, attr_s, br_s, layer):
                gpool_ps = None
                if layer == 2:
                    gpool_ps = psG.tile([G, HC], f32, name="gpool_ps")
                ca = cb = 0
                call_i = [0]
                for b in range(cfg.nblk):
                    tb = cfg.tb[b]
                    tba, tbb = cfg.tba[b], cfg.tbb[b]
                    c0 = cfg.col0[b]
                    ot_t = stripp.tile([P, tb * P], dt.float8e4, tag="ot")
                    nc.sync.dma_start(out=ot_t[:], in_=otsp[:, c0 * P : (c0 + tb) * P])
                    ott_t = stripp.tile([P, tb * P], dt.float8e4, tag="ott")
                    nc.sync.dma_start(out=ott_t[:], in_=ottsp[:, c0 * P : (c0 + tb) * P])
                    xr_blk = metap.tile([P, HC], bf, tag="xrb")
                    nc.sync.dma_start(out=xr_blk[:], in_=xr_tab[b * P : (b + 1) * P, :])

                    def chunked_gather(dst, dst_t0, n_tiles, table, idxt):
                        for q0 in range(0, n_tiles, CH):
                            q1 = min(q0 + CH, n_tiles)
                            uc = cfg.ucnt[call_i[0]]
                            call_i[0] += 1
                            nc.gpsimd.dma_gather(
                                out_ap=dst[:, dst_t0 + q0 : dst_t0 + q1, :],
                                in_ap=table,
                                idxs_ap=idxt[:, q0 * 8 : q1 * 8],
                                num_idxs=(q1 - q0) * P, num_idxs_reg=uc,
                                elem_size=HC,
                            )

                    gxl = gbufp.tile([P, cfg.tbmax, HC], bf, tag="gxl")
                    if tba:
                        ixa_t = metap.tile([P, tba * 8], dt.int16, tag="ixa")
                        nc.sync.dma_start(out=ixa_t[:], in_=ixa[:, ca : ca + tba * 8])
                        chunked_gather(
                            gxl, 0, tba,
                            xl_ext[0:MAXI16, :] if cfg.npiece == 2 else xl_ext[:],
                            ixa_t,
                        )
                    if tbb:
                        ixb_t = metap.tile([P, tbb * 8], dt.int16, tag="ixb")
                        nc.sync.dma_start(out=ixb_t[:], in_=ixb[:, cb : cb + tbb * 8])
                        chunked_gather(
                            gxl, tba, tbb, xl_ext[MAXI16 : cfg.capext, :], ixb_t
                        )

                    # u = one-hot @ xr_block + I @ xl  (PSUM, per tile-pair)
                    ft = workp.tile([P, cfg.tbmax, HC], bf, tag="ft")
                    for q0 in range(0, tb, 2):
                        w = min(2, tb - q0)
                        psu = psU.tile([P, 2, HC], f32, tag="u")
                        for i in range(w):
                            t = q0 + i
                            nc.tensor.matmul(
                                psu[:, i, :], lhsT=ott_t[:, t * P : (t + 1) * P],
                                rhs=xr_blk[:], start=True, stop=False,
                            )
                            nc.tensor.matmul(
                                psu[:, i, :], lhsT=ident8_s[:], rhs=gxl[:, t, :],
                                start=False, stop=True,
                            )
                        nc.scalar.activation(
                            out=ft[:, q0 : q0 + w, :], in_=psu[:, 0:w, :],
                            func=AF.Prelu, alpha=0.2,
                        )

                    nc.vector.tensor_tensor(
                        out=ft[:, 0:tb, :],
                        in0=ft[:, 0:tb, :],
                        in1=attr_s[:].unsqueeze(1).broadcast_to([P, tb, HC]),
                        op=A_.mult,
                    )
                    v = ft[:, 0:tb, :].rearrange("p t (h c) -> p (t h) c", h=H)
                    t1 = workp.tile([P, tb * H, 16], bf, tag="t1")
                    nc.vector.tensor_tensor(out=t1[:], in0=v[:, :, 0:16], in1=v[:, :, 16:32], op=A_.add)
                    t2 = workp.tile([P, tb * H, 8], bf, tag="t2")
                    nc.vector.tensor_tensor(out=t2[:], in0=t1[:, :, 0:8], in1=t1[:, :, 8:16], op=A_.add)
                    t3 = workp.tile([P, tb * H, 4], bf, tag="t3")
                    nc.vector.tensor_tensor(out=t3[:], in0=t2[:, :, 0:4], in1=t2[:, :, 4:8], op=A_.add)
                    t4 = workp.tile([P, tb * H, 2], bf, tag="t4")
                    nc.vector.tensor_tensor(out=t4[:], in0=t3[:, :, 0:2], in1=t3[:, :, 2:4], op=A_.add)
                    lg = workp.tile([P, tb * H], bf, tag="lg")
                    nc.vector.tensor_tensor(
                        out=lg[:].unsqueeze(2), in0=t4[:, :, 0:1], in1=t4[:, :, 1:2], op=A_.add
                    )
                    ex = workp.tile([P, tb * H], bf, tag="ex")
                    nc.scalar.activation(out=ex[:], in_=lg[:], func=AF.Exp)

                    # msgext[:, :, 0:HC] = xl * exp(logit) (head-broadcast);
                    # msgext[:, :, HC:] = exp(logit) per head (denominator)
                    msgext = workp.tile([P, tb, HCD], bf, tag="msg")
                    exv = ex[:].rearrange("p (t h) -> p t h", h=H)
                    nc.vector.tensor_tensor(
                        out=msgext[:, :, 0:HC].rearrange("p t (h c) -> p t h c", h=H),
                        in0=gxl[:, 0:tb, :].rearrange("p t (h c) -> p t h c", h=H),
                        in1=exv.unsqueeze(3).to_broadcast([P, tb, H, C]),
                        op=A_.mult,
                    )
                    nc.vector.tensor_copy(out=msgext[:, :, HC:HCD], in_=exv)

                    acc = psS.tile([P, HCD], f32, tag="s")
                    for t in range(tb):
                        nc.tensor.matmul(
                            acc[:], lhsT=ot_t[:, t * P : (t + 1) * P],
                            rhs=msgext[:, t, :],
                            start=(t == 0), stop=(t == tb - 1),
                        )

                    denom = smallp.tile([P, H], f32, tag="denom")
                    nc.vector.tensor_scalar(
                        out=denom[:], in0=acc[:, HC:HCD], scalar1=1e-20, scalar2=None,
                        op0=A_.max,
                    )
                    rec = smallp.tile([P, H], f32, tag="rec")
                    nc.vector.reciprocal(out=rec[:], in_=denom[:])
                    hsc = smallp.tile([P, HC], bf, tag="hsc")
                    nc.vector.tensor_tensor(
                        out=hsc[:].rearrange("p (h c) -> p h c", h=H),
                        in0=acc[:, 0:HC].rearrange("p (h c) -> p h c", h=H),
                        in1=rec[:].to_broadcast([P, H, C]),
                        op=A_.mult,
                    )
                    hfin = smallp.tile([P, HC], bf, tag="hfin")
                    nc.vector.tensor_tensor(out=hfin[:], in0=hsc[:], in1=br_s[:], op=A_.add)
                    hout = smallp.tile([P, HC], bf, tag="hout")
                    nc.scalar.activation(out=hout[:], in_=hfin[:], func=AF.Relu)

                    if layer == 1:
                        # transpose h and immediately build layer-2 node tables
                        tps = []
                        for kt in range(2):
                            tp = psA.tile([P, P], bf, tag="a")
                            nc.tensor.transpose(
                                out=tp[:], in_=hout[:, kt * P : (kt + 1) * P],
                                identity=ident_s[:],
                            )
                            tpev = smallp.tile([P, P], bf, tag="htps")
                            nc.scalar.activation(out=tpev[:], in_=tp[:], func=AF.Copy)
                            tps.append(tpev)
                        for W_s, tab in ((Wl2_s, xl2_own), (Wr2_s, xr2_tab)):
                            ps = psA.tile([P, HC], f32, tag="a")
                            for kt in range(2):
                                nc.tensor.matmul(
                                    ps[:], lhsT=tps[kt][:], rhs=W_s[kt][:],
                                    start=(kt == 0), stop=(kt == 1),
                                )
                            ev = smallp.tile([P, HC], bf, tag="tabev")
                            nc.scalar.activation(out=ev[:], in_=ps[:], func=AF.Copy)
                            nc.sync.dma_start(
                                out=tab[b * P : (b + 1) * P, :], in_=ev[:]
                            )
                    else:
                        gsel_blk = metap.tile([P, G], bf, tag="gselb")
                        nc.sync.dma_start(
                            out=gsel_blk[:], in_=gselp[b * P : (b + 1) * P, :]
                        )
                        nc.tensor.matmul(
                            gpool_ps[:], lhsT=gsel_blk[:], rhs=hout[:],
                            start=(b == 0), stop=(b == cfg.nblk - 1),
                        )
                    ca += tba * 8
                    cb += tbb * 8
                return gpool_ps

            edge_layer(xl1_ext, xr1_tab, att1r_s, b1r_s, layer=1)
            nc.gpsimd.collective_compute(
                "AllGather", A_.bypass, replica_groups=groups,
                ins=[xl2_own.opt()], outs=[xl2_ext.opt()],
            )
            gpool_ps = edge_layer(xl2_ext, xr2_tab, att2r_s, b2r_s, layer=2)

            # ================= pool + MLP =================
            gsum = smallp.tile([G, HC], f32, tag="gsum")
            nc.scalar.activation(out=gsum[:], in_=gpool_ps[:], func=AF.Copy)
            nc.sync.dma_start(out=gp_in[:], in_=gsum[:])
            nc.gpsimd.collective_compute(
                "AllReduce", A_.add, replica_groups=groups,
                ins=[gp_in.opt()], outs=[gp_out.opt()],
            )
            gsum2 = smallp.tile([G, HC], f32, tag="gsum2")
            nc.sync.dma_start(out=gsum2[:], in_=gp_out[:])
            gmean = smallp.tile([G, HC], bf, tag="gmean")
            nc.vector.tensor_scalar(
                out=gmean[:], in0=gsum2[:], scalar1=crecip_s[:, 0:1], scalar2=None,
                op0=A_.mult,
            )
            gT = []
            for kt in range(2):
                tp = psA.tile([P, G], bf, tag="a")
                nc.tensor.transpose(
                    out=tp[:], in_=gmean[:, kt * P : (kt + 1) * P], identity=ident_s[:]
                )
                gkt = smallp.tile([P, G], bf, tag="gT", name=f"gT{kt}")
                nc.scalar.activation(out=gkt[:], in_=tp[:], func=AF.Copy)
                gT.append(gkt)
            lin_ps = psS.tile([G, 64], f32, tag="s")
            for kt in range(2):
                nc.tensor.matmul(
                    lin_ps[:], lhsT=gT[kt][:], rhs=Wlin1_s[kt][:],
                    start=(kt == 0), stop=(kt == 1),
                )
            lin = smallp.tile([G, 64], f32, tag="lin")
            nc.vector.tensor_tensor(out=lin[:], in0=lin_ps[:], in1=blin1r_s[:], op=A_.add)
            glu = smallp.tile([G, P], bf, tag="glu")
            nc.scalar.activation(out=glu[:, 0:64], in_=lin[:], func=AF.Relu)
            nc.vector.tensor_copy(out=glu[:, 64:67], in_=ub_s[:])
            nc.gpsimd.memset(glu[:, 67:P], 0.0)
            tp = psA.tile([P, G], bf, tag="a")
            nc.tensor.transpose(out=tp[:], in_=glu[:], identity=ident_s[:])
            gluT = smallp.tile([P, G], bf, tag="gluT")
            nc.scalar.activation(out=gluT[:], in_=tp[:], func=AF.Copy)
            out_ps = psS.tile([G, 1], f32, tag="s")
            nc.tensor.matmul(
                out_ps[:], lhsT=gluT[0:67, :], rhs=Wout_s[:], start=True, stop=True
            )
            outs = smallp.tile([G, 1], f32, tag="outs")
            nc.vector.tensor_tensor(out=outs[:], in0=out_ps[:], in1=boutr_s[:], op=A_.add)
            nc.sync.dma_start(out=out_g[:], in_=outs[:])

    nc.compile()
    _split_waits(nc)
    return nc


# ---------------------------------------------------------------------------
# Entry point
# ---------------------------------------------------------------------------


def kernel(**inputs):
    import os

    from concourse.bass_utils import run_bass_kernel_spmd

    x = np.asarray(inputs["x"], np.float32)
    edge_index = np.asarray(inputs["edge_index"], np.int64)
    batch = np.asarray(inputs["batch"], np.int64)
    u = np.asarray(inputs["u"], np.float32)
    weights = {
        k: np.asarray(inputs[k], np.float32)
        for k in ("Wl1", "Wr1", "att1", "b1", "Wl2", "Wr2", "att2", "b2",
                  "W_lin1", "b_lin1", "W_out", "b_out")
    }
    percore, nblk, tba, tbb, ucnt, node2row = _plan_blocks(edge_index, N, NCORES)
    cfg = Cfg(N, NCORES, nblk, tba, tbb, ucnt)
    maps = _prep(x, edge_index, batch, u, weights, cfg, percore, node2row)
    nc = _build(cfg, in_dim=x.shape[1])
    trace = bool(os.environ.get("KERNEL_TRACE"))
    try:
        res = run_bass_kernel_spmd(nc, maps, list(range(NCORES)), trace=trace)
    except ModuleNotFoundError:
        res = run_bass_kernel_spmd(nc, maps, list(range(NCORES)))
    if trace and getattr(res, "exec_time_ns", None) is not None:
        print(f"HW exec time: {res.exec_time_ns} ns")
    return res.results[0]["out_g"].reshape(G).astype(np.float32)
